# revision 33
# baseline (speedup 1.0000x reference)
"""CT self-attention (causal + 2 future frames) for Trainium2, 8 NeuronCores.

Sharding: batch (4-way) x head-group (2-way): core c = 2*b + g handles batch b,
heads [8g, 8g+8). Each core computes its QKV projection slice, banded
attention for its 8 heads, and a partial output projection; the host sums the
two partial outputs per batch and adds the (host-folded) biases.

All matmuls run in bf16 (1 col/cycle @ 2.4 GHz warm) with fp32 PSUM.
Schedule is built for PE density (HAM stays at full clock):
  - startup: f-major sweep accumulating V t0-4 + K0 (first 2 blocks) + Q0
    (block 0) in 8 PSUM banks while the x^T / w DMAs stream in, then the
    remaining K0/Q0 blocks
  - attention for block q5=0 interleaves the rest of the QKV projection
    (V t5-15, K1-3, Q1-3) between head pairs as PE gap filler
  - attention: scores S_T = K^T-tile.T @ Q (2 heads row-tiled), CT mask via
    accumulating -1e9*I @ MQ matmul, exp on ScalarE (scale 1/8, key-padding
    bias), AV with a ones column on V (denominator on partition 64)
  - normalize per head pair: 2 denominator rows -> [2,512] fast reciprocal,
    one [2,128] selector matmul broadcasts both heads' reciprocals across
    128 partitions, 2 DVE mults into AT; deferred into the next block's
    stream (immediate for the last block)
  - output projection per 128-query tile once all 4 pairs are normalized
"""
import math
from contextlib import ExitStack

import numpy as np

B, T, D, H = 4, 2048, 1024, 16
HD = D // H            # 64
L = 2                  # max_future_frames
NCORES = 8
HPG = 8                # heads per group/core
NPAIR = 4              # head pairs per core
FCH = 8                # feature chunks (D / 128)
TQ5 = 4                # 512-wide query tiles
NKT = 16               # 128-wide key tiles
NEG = -1.0e9

_BUILT = {}


def _build_nc():
    import concourse.tile as tile
    from concourse import bacc, mybir

    dt = mybir.dt
    f32, f32r, bf16 = dt.float32, dt.float32r, dt.bfloat16
    Exp = mybir.ActivationFunctionType.Exp
    MUL = mybir.AluOpType.mult
    ADD = mybir.AluOpType.add

    nc = bacc.Bacc(None, target_bir_lowering=False)
    xT_d = nc.dram_tensor("xT", [FCH, 128, T], bf16, kind="ExternalInput")
    wqkvT_d = nc.dram_tensor("wqkvT", [FCH, 128, 3 * 512], bf16, kind="ExternalInput")
    woutT_d = nc.dram_tensor("woutT", [NPAIR, 128, D], bf16, kind="ExternalInput")
    bq_d = nc.dram_tensor("bq", [128, NPAIR], f32, kind="ExternalInput")
    bk_d = nc.dram_tensor("bk", [128, NPAIR], f32, kind="ExternalInput")
    kpb_d = nc.dram_tensor("kpb", [128, NKT], f32, kind="ExternalInput")
    mq_d = nc.dram_tensor("mq", [128, 5, 512], bf16, kind="ExternalInput")
    mk_d = nc.dram_tensor("mk", [128, 128], bf16, kind="ExternalInput")
    sel2_d = nc.dram_tensor("sel2", [2, 128], bf16, kind="ExternalInput")
    vones_d = nc.dram_tensor("vones", [128, NKT * HPG], bf16, kind="ExternalInput")
    out_d = nc.dram_tensor("out_part", [T, D], f32, kind="ExternalOutput")

    with tile.TileContext(nc) as tc, \
         nc.allow_low_precision(reason="bf16 matmul fast path"), \
         ExitStack() as top:
        pers = top.enter_context(tc.tile_pool(name="pers", bufs=1))
        xT_sb = pers.tile([128, FCH, T], bf16, name="xT_sb")
        wk_sb = pers.tile([128, FCH, 3 * 512], bf16, name="wk_sb")
        QT = pers.tile([128, NPAIR, T], bf16, name="QT")
        KT = pers.tile([128, NPAIR, T], bf16, name="KT")
        Vt = pers.tile([128, NKT, HPG, HD + 1], bf16, name="Vt")
        AT = pers.tile([128, NPAIR, T], bf16, name="AT")
        wo_sb = pers.tile([128, NPAIR, D], bf16, name="wo_sb")
        mq_sb = pers.tile([128, 5, 512], bf16, name="mq_sb")
        mk_sb = pers.tile([128, 128], bf16, name="mk_sb")
        kp_sb = pers.tile([128, NKT], f32, name="kp_sb")
        bq_sb = pers.tile([128, NPAIR], f32, name="bq_sb")
        bk_sb = pers.tile([128, NPAIR], f32, name="bk_sb")
        sel2_sb = pers.tile([2, 128], bf16, name="sel2_sb")
        vones_sb = pers.tile([128, NKT * HPG], bf16, name="vones_sb")
        # priority-sliced input loads: the f-major startup sweep needs, per f,
        # only the V weight columns, the QK weight columns, and the first
        # 1024 x columns -- stream those first (alternating both queues),
        # defer the upper x columns (used by fillers) and w_out
        nc.sync.dma_start(wk_sb[:, 0, 1024:1536], wqkvT_d[0, :, 1024:1536])
        nc.gpsimd.dma_start(xT_sb[:, 0, 0:1024], xT_d[0, :, 0:1024])
        nc.sync.dma_start(wk_sb[:, 0, 0:1024], wqkvT_d[0, :, 0:1024])
        nc.gpsimd.dma_start(mq_sb[:], mq_d[:])
        nc.gpsimd.dma_start(mk_sb[:], mk_d[:])
        nc.gpsimd.dma_start(kp_sb[:], kpb_d[:])
        nc.gpsimd.dma_start(bq_sb[:], bq_d[:])
        nc.gpsimd.dma_start(bk_sb[:], bk_d[:])
        nc.gpsimd.dma_start(sel2_sb[:], sel2_d[:])
        nc.gpsimd.dma_start(vones_sb[:], vones_d[:])
        for f in range(1, FCH):
            nc.sync.dma_start(wk_sb[:, f, :], wqkvT_d[f])
            nc.gpsimd.dma_start(xT_sb[:, f, 0:1024], xT_d[f, :, 0:1024])
        for f in range(FCH):
            (nc.sync if f % 2 else nc.gpsimd).dma_start(
                xT_sb[:, f, 1024:2048], xT_d[f, :, 1024:2048])
        for cchunk in range(NPAIR):
            (nc.sync if cchunk % 2 else nc.gpsimd).dma_start(
                wo_sb[:, cchunk, :], woutT_d[cchunk])
        nc.vector.tensor_copy(
            Vt[:, :, :, HD],
            vones_sb[:].rearrange("p (a b) -> p a b", a=NKT))

        def wQ(f, p):
            return wk_sb[:, f, 128 * p:128 * (p + 1)]

        def wK(f, p):
            return wk_sb[:, f, 512 + 128 * p:512 + 128 * (p + 1)]

        def wV(f):
            return wk_sb[:, f, 1024:1536]

        def v_copy(pv, t):
            nc.vector.tensor_copy(
                Vt[:, t, :, 0:HD],
                pv[:].rearrange("p (h d) -> p h d", h=HPG))

        def qk_store(pqk, tgt, t5):
            pair = tgt % 4
            dst = (QT if tgt < 4 else KT)[:, pair, t5 * 512:(t5 + 1) * 512]
            bias = (bq_sb if tgt < 4 else bk_sb)[:, pair:pair + 1]
            nc.vector.tensor_scalar(dst, pqk[:], bias, None, ADD)

        # ---- startup: f-major sweep (PE works while x^T still streams) ----
        with tc.tile_pool(name="psUp", bufs=1, space="PSUM") as psUp:
            pvs = [psUp.tile([128, 512], f32, name=f"pv{t}", tag=f"u{t}")
                   for t in range(5)]
            pk0 = psUp.tile([128, 512], f32, name="pk0", tag="u5")
            pk1 = psUp.tile([128, 512], f32, name="pk1", tag="u6")
            pq0 = psUp.tile([128, 512], f32, name="pq0", tag="u7")
            for f in range(FCH):
                st = dict(start=(f == 0), stop=(f == FCH - 1))
                for t in range(5):
                    nc.tensor.matmul(pvs[t][:], xT_sb[:, f, t * 128:(t + 1) * 128],
                                     wV(f), **st)
                nc.tensor.matmul(pk0[:], wK(f, 0), xT_sb[:, f, 0:512], **st)
                nc.tensor.matmul(pk1[:], wK(f, 0), xT_sb[:, f, 512:1024], **st)
                nc.tensor.matmul(pq0[:], wQ(f, 0), xT_sb[:, f, 0:512], **st)
            for t in range(5):
                v_copy(pvs[t], t)
            nc.vector.tensor_scalar(KT[:, 0, 0:512], pk0[:], bk_sb[:, 0:1], None, ADD)
            nc.vector.tensor_scalar(KT[:, 0, 512:1024], pk1[:], bk_sb[:, 0:1], None, ADD)
            nc.vector.tensor_scalar(QT[:, 0, 0:512], pq0[:], bq_sb[:, 0:1], None, ADD)

        # ---- attention (with projection filler inside block 0) ----
        with tc.tile_pool(name="eps", bufs=3) as epool, \
             tc.tile_pool(name="nsb", bufs=1) as nsb, \
             tc.tile_pool(name="avp", bufs=1) as avp, \
             tc.tile_pool(name="osb", bufs=2) as osb, \
             tc.tile_pool(name="psAv", bufs=1, space="PSUM") as psAv, \
             tc.tile_pool(name="psSc", bufs=2, space="PSUM") as psSc, \
             tc.tile_pool(name="psX", bufs=1, space="PSUM") as psX:

            nx = [0]

            def emitQK(tgt, t5):
                pqk = psX.tile([128, 512], f32, name="pqk", tag=f"x{nx[0] % 2}")
                nx[0] += 1
                w = wQ if tgt < 4 else wK
                for f in range(FCH):
                    nc.tensor.matmul(pqk[:], w(f, tgt % 4),
                                     xT_sb[:, f, t5 * 512:(t5 + 1) * 512],
                                     start=(f == 0), stop=(f == FCH - 1))
                qk_store(pqk, tgt, t5)

            def emitV(t):
                pv = psX.tile([128, 512], f32, name="pv", tag=f"x{nx[0] % 2}")
                nx[0] += 1
                for f in range(FCH):
                    nc.tensor.matmul(pv[:], xT_sb[:, f, t * 128:(t + 1) * 128],
                                     wV(f), start=(f == 0), stop=(f == FCH - 1))
                v_copy(pv, t)

            # remaining startup QK blocks (t5-major, 2-bank pipeline)
            rest = [(4, 2), (4, 3), (0, 1), (0, 2), (0, 3)]
            fillers = [
                [("V", t) for t in range(5, 9)] + [(5, t5) for t5 in range(4)]
                + [(1, t5) for t5 in range(4)],
                [("V", t) for t in range(9, 13)] + [(6, t5) for t5 in range(4)]
                + [(2, t5) for t5 in range(4)],
                [("V", t) for t in range(13, 16)] + [(7, t5) for t5 in range(4)]
                + [(3, t5) for t5 in range(4)],
                [],
            ]
            for tgt, t5 in rest:
                emitQK(tgt, t5)

            norm_pend = {}

            def emit_normalize_pair(q5, p):
                av2, recp = norm_pend.pop((q5, p))
                qs = slice(q5 * 512, (q5 + 1) * 512)
                bc = psX.tile([128, 512], f32, name="bc", tag="x0")
                if isinstance(recp, tuple):
                    # tail fast path: two per-head broadcasts from separate
                    # [1, 512] reciprocal rows
                    nc.tensor.matmul(bc[0:64, :], sel2_sb[0:1, 0:64],
                                     recp[0][:], start=True, stop=True,
                                     tile_position=(0, 0))
                    nc.tensor.matmul(bc[64:128, :], sel2_sb[0:1, 0:64],
                                     recp[1][:], start=True, stop=True,
                                     tile_position=(0, 64))
                else:
                    nc.tensor.matmul(bc[:], sel2_sb[:], recp[:],
                                     start=True, stop=True)
                nc.vector.tensor_tensor(AT[0:64, p, qs], av2[0:64, 0, :],
                                        bc[0:64, :], MUL)
                nc.vector.tensor_tensor(AT[64:128, p, qs], av2[0:64, 1, :],
                                        bc[64:128, :], MUL)

            def emit_proj(q5):
                for tq in range(4):
                    t = 4 * q5 + tq
                    tsl = slice(t * 128, (t + 1) * 128)
                    po0 = psX.tile([128, 512], f32, name="po0", tag="x0")
                    po1 = psX.tile([128, 512], f32, name="po1", tag="x1")
                    for cchunk in range(NPAIR):
                        lhsT = AT[:, cchunk, tsl]
                        nc.tensor.matmul(po0[:], lhsT, wo_sb[:, cchunk, 0:512],
                                         start=(cchunk == 0), stop=(cchunk == 3))
                        nc.tensor.matmul(po1[:], lhsT, wo_sb[:, cchunk, 512:1024],
                                         start=(cchunk == 0), stop=(cchunk == 3))
                    ot = osb.tile([128, D], f32, name="ot", tag="ot")
                    nc.vector.tensor_copy(ot[:, 0:512], po0[:])
                    nc.vector.tensor_copy(ot[:, 512:1024], po1[:])
                    (nc.sync if t % 2 else nc.gpsimd).dma_start(
                        out_d[tsl, :], ot[:])

            # Flat software pipeline across pairs/blocks: each pair's last AV
            # is deferred past the next pair's first score group so the PE
            # never drains while ScalarE finishes the last exp.
            pend_av = [None]

            def flush_av():
                if pend_av[0] is not None:
                    pend_av[0]()
                    pend_av[0] = None

            def finish_pair(q5, p, avA, avB):
                av2 = avp.tile([HD + 1, 2, 512], f32,
                               name=f"av2_{p}", tag=f"avp{p}")
                if q5 == TQ5 - 1 and p == 3:
                    # tail-critical pair: per-head base-0 staging copies from
                    # the PSUM ones-rows, then reciprocal + cast per head (no
                    # DMA latency in the chain)
                    dA = nsb.tile([1, 512], f32, name="dA", tag=f"dp{p}")
                    nc.vector.tensor_copy(dA[:], avA[64:65, :])
                    dB = nsb.tile([1, 512], f32, name="dB", tag="dx")
                    nc.vector.tensor_copy(dB[:], avB[64:65, :])
                    rcA = nsb.tile([1, 512], f32, name="rcA", tag=f"di{p}")
                    nc.vector.reciprocal_approx_fast(rcA[:], dA[:])
                    rcB = nsb.tile([1, 512], f32, name="rcB", tag=f"dp{p}")
                    nc.vector.reciprocal_approx_fast(rcB[:], dB[:])
                    rbA = nsb.tile([1, 512], bf16, name="rbA", tag=f"rc{p}")
                    nc.gpsimd.tensor_copy(rbA[:], rcA[:])
                    rbB = nsb.tile([1, 512], bf16, name="rbB", tag="rbx")
                    nc.gpsimd.tensor_copy(rbB[:], rcB[:])
                    nc.vector.tensor_copy(av2[:, 0, :], avA[:])
                    nc.vector.tensor_copy(av2[:, 1, :], avB[:])
                    norm_pend[(q5, p)] = (av2, (rbA, rbB))
                    return
                # AV out of PSUM into one combined tile (slot-freeing copies
                # first), then both denominator rows gathered with a single
                # SBUF->SBUF DMA (DVE can't address partition base 1) for one
                # batched reciprocal + cast
                nc.vector.tensor_copy(av2[:, 0, :], avA[:])
                nc.vector.tensor_copy(av2[:, 1, :], avB[:])
                d2 = nsb.tile([2, 512], f32, name="d2", tag=f"dp{p}")
                nc.gpsimd.dma_start(d2[:], av2[64:65, :, :])
                rc32 = nsb.tile([2, 512], f32, name="rc32", tag=f"di{p}")
                nc.vector.reciprocal_approx_fast(rc32[:], d2[:])
                recp = nsb.tile([2, 512], bf16, name="recp", tag=f"rc{p}")
                nc.gpsimd.tensor_copy(recp[:], rc32[:])
                norm_pend[(q5, p)] = (av2, recp)

            for q5 in range(TQ5):
                nkt = min(4 * q5 + 5, NKT)
                q5s = q5 * 512
                for p in range(NPAIR):
                    avA = psAv.tile([HD + 1, 512], f32, name="avA", tag="avA")
                    avB = psAv.tile([HD + 1, 512], f32, name="avB", tag="avB")
                    for kt in range(nkt):
                        ks = slice(kt * 128, (kt + 1) * 128)
                        off = kt - 4 * q5
                        masked = off >= 0
                        # masked tiles only affect queries >= q0
                        q0 = max(0, 128 * off - L) if masked else 0
                        qs = slice(q5s + q0, q5s + 512)
                        sc2 = psSc.tile([128, 2, 512], f32, name="sc2", tag="sc2")
                        nc.tensor.matmul(sc2[:, 0, q0:512],
                                         KT[0:64, p, ks], QT[0:64, p, qs],
                                         start=True, stop=not masked,
                                         tile_position=(0, 0))
                        nc.tensor.matmul(sc2[:, 1, q0:512],
                                         KT[64:128, p, ks], QT[64:128, p, qs],
                                         start=True, stop=not masked,
                                         tile_position=(64, 0))
                        if masked:
                            m1 = min(512, 128 * off + 126)
                            nc.tensor.matmul(sc2[:, 0, q0:m1], mk_sb[:],
                                             mq_sb[:, off, q0:m1],
                                             start=False, stop=True,
                                             skip_group_check=True)
                            nc.tensor.matmul(sc2[:, 1, q0:m1], mk_sb[:],
                                             mq_sb[:, off, q0:m1],
                                             start=False, stop=True,
                                             skip_group_check=True)
                        flush_av()
                        if kt == 1:
                            # deferred bookkeeping once the pipeline is primed:
                            # block-delayed normalize of (q5-1, p), plus
                            # pair-delayed normalize inside the last block
                            if q5 >= 1 and (q5 - 1, p) in norm_pend:
                                emit_normalize_pair(q5 - 1, p)
                                if p == 3:
                                    emit_proj(q5 - 1)
                        if kt == 2 and q5 == TQ5 - 1 and p >= 1 and \
                                (q5, p - 1) in norm_pend:
                            emit_normalize_pair(q5, p - 1)
                        e2 = epool.tile([128, 2, 512], bf16, name="e2", tag="e2")
                        nc.scalar.activation(e2[:, :, q0:512], sc2[:, :, q0:512],
                                             Exp, bias=kp_sb[:, kt:kt + 1],
                                             scale=1.0 / math.sqrt(HD))

                        def mk_av(kt=kt, e2=e2, q0=q0, avA=avA, avB=avB,
                                  p=p, nkt=nkt, q5=q5):
                            nc.tensor.matmul(avA[0:65, q0:512],
                                             Vt[:, kt, 2 * p, :],
                                             e2[:, 0, q0:512],
                                             start=(kt == 0), stop=(kt == nkt - 1),
                                             skip_group_check=True)
                            nc.tensor.matmul(avB[0:65, q0:512],
                                             Vt[:, kt, 2 * p + 1, :],
                                             e2[:, 1, q0:512],
                                             start=(kt == 0), stop=(kt == nkt - 1),
                                             skip_group_check=True)
                            if kt == nkt - 1:
                                finish_pair(q5, p, avA, avB)
                        pend_av[0] = mk_av
                    # projection filler inside block 0 keeps the PE warm
                    if q5 == 0:
                        for j, item in enumerate(fillers[p]):
                            if item[0] == "V":
                                emitV(item[1])
                            else:
                                emitQK(item[0], item[1])
                            if j == 0:
                                flush_av()
            # drain: last pair's AV, its normalize, last projection
            flush_av()
            emit_normalize_pair(TQ5 - 1, 3)
            emit_proj(TQ5 - 1)

    nc.finalize()
    return nc


def _host_inputs(x, key_padding_mask, w_qkv, b_qkv, w_out):
    """Per-core input dicts."""
    import ml_dtypes

    f32 = np.float32
    bf = ml_dtypes.bfloat16
    # masks (shared across cores)
    j = np.arange(128)[:, None]
    q = np.arange(512)[None, :]
    mq = np.zeros((128, 5, 512), f32)
    for off in range(5):
        mq[:, off, :] = (128 * off + j > q + L).astype(f32)
    mq = mq.astype(bf)
    mk = (NEG * np.eye(128, dtype=f32)).astype(bf)
    vones = np.ones((128, NKT * HPG), bf)
    sel2 = np.zeros((2, 128), f32)
    sel2[0, 0:64] = 1.0
    sel2[1, 64:128] = 1.0
    sel2 = sel2.astype(bf)

    in_maps = []
    for c in range(NCORES):
        b, g = divmod(c, 2)
        # channel rows for this group's Q/K (pairs of heads -> 128 rows each)
        qrows = np.concatenate(
            [w_qkv[64 * (8 * g + 2 * p):64 * (8 * g + 2 * p) + 128] for p in range(NPAIR)])
        krows = np.concatenate(
            [w_qkv[D + 64 * (8 * g + 2 * p):D + 64 * (8 * g + 2 * p) + 128] for p in range(NPAIR)])
        vrows = w_qkv[2 * D + 512 * g:2 * D + 512 * g + 512]
        w_all = np.concatenate([qrows, krows, vrows], 0)          # [1536, 1024]
        wqkvT = np.ascontiguousarray(w_all.T).reshape(FCH, 128, 3 * 512)
        bq = np.stack(
            [b_qkv[64 * (8 * g + 2 * p):64 * (8 * g + 2 * p) + 128] for p in range(NPAIR)], 1)
        bk = np.stack(
            [b_qkv[D + 64 * (8 * g + 2 * p):D + 64 * (8 * g + 2 * p) + 128] for p in range(NPAIR)], 1)
        xT = np.ascontiguousarray(x[b].T).reshape(FCH, 128, T)
        woutT = np.ascontiguousarray(w_out.T[512 * g:512 * g + 512]).reshape(NPAIR, 128, D)
        kpb = np.ascontiguousarray(
            (NEG * key_padding_mask[b].astype(f32)).reshape(NKT, 128).T)
        in_maps.append({
            "xT": xT.astype(bf), "wqkvT": wqkvT.astype(bf),
            "woutT": woutT.astype(bf),
            "bq": bq.astype(f32), "bk": bk.astype(f32), "kpb": kpb.astype(f32),
            "mq": mq, "mk": mk, "vones": vones, "sel2": sel2,
        })
    return in_maps


def kernel(x, key_padding_mask, w_qkv, b_qkv, w_out, b_out):
    from concourse.bass_utils import run_bass_kernel_spmd

    x = np.asarray(x, np.float32)
    key_padding_mask = np.asarray(key_padding_mask)
    w_qkv = np.asarray(w_qkv, np.float32)
    b_qkv = np.asarray(b_qkv, np.float32)
    w_out = np.asarray(w_out, np.float32)
    b_out = np.asarray(b_out, np.float32)

    if "nc" not in _BUILT:
        _BUILT["nc"] = _build_nc()
    nc = _BUILT["nc"]

    in_maps = _host_inputs(x, key_padding_mask, w_qkv, b_qkv, w_out)
    res = run_bass_kernel_spmd(nc, in_maps, core_ids=list(range(NCORES)))
    out = np.empty((B, T, D), np.float32)
    for b in range(B):
        out[b] = res.results[2 * b]["out_part"] + res.results[2 * b + 1]["out_part"]
    # host-folded biases: b_out plus the V-bias pushed through the projection
    bv = b_qkv[2 * D:3 * D]
    out += (b_out + bv @ w_out.T)[None, None, :].astype(np.float32)
    return out


# revision 34
# speedup vs baseline: 1.0824x; 1.0824x over previous
"""CT self-attention (causal + 2 future frames) for Trainium2, 8 NeuronCores.

Sharding: batch (4-way) x head-group (2-way): core c = 2*b + g handles batch b,
heads [8g, 8g+8). Each core computes its QKV projection slice, banded
attention for its 8 heads, and a partial output projection; the host sums the
two partial outputs per batch and adds the (host-folded) biases.

All matmuls run in bf16 (1 col/cycle @ 2.4 GHz warm) with fp32 PSUM.
Schedule is built for PE density (HAM stays at full clock):
  - startup: f-major sweep accumulating V t0-4 + K0 (first 2 blocks) + Q0
    (block 0) in 8 PSUM banks while the x^T / w DMAs stream in, then the
    remaining K0/Q0 blocks
  - attention for block q5=0 interleaves the rest of the QKV projection
    (V t5-15, K1-3, Q1-3) between head pairs as PE gap filler
  - attention: scores S_T = K^T-tile.T @ Q (2 heads row-tiled), CT mask via
    accumulating -1e9*I @ MQ matmul, exp on ScalarE (scale 1/8, key-padding
    bias), AV with a ones column on V (denominator on partition 64)
  - normalize per head pair: 2 denominator rows -> [2,512] fast reciprocal,
    one [2,128] selector matmul broadcasts both heads' reciprocals across
    128 partitions, 2 DVE mults into AT; deferred into the next block's
    stream (immediate for the last block)
  - output projection per 128-query tile once all 4 pairs are normalized
"""
import math
from contextlib import ExitStack

import numpy as np

B, T, D, H = 4, 2048, 1024, 16
HD = D // H            # 64
L = 2                  # max_future_frames
NCORES = 8
HPG = 8                # heads per group/core
NPAIR = 4              # head pairs per core
FCH = 8                # feature chunks (D / 128)
TQ5 = 4                # 512-wide query tiles
NKT = 16               # 128-wide key tiles
NEG = -1.0e9

_BUILT = {}


def _build_nc():
    import concourse.tile as tile
    from concourse import bacc, mybir

    dt = mybir.dt
    f32, f32r, bf16 = dt.float32, dt.float32r, dt.bfloat16
    Exp = mybir.ActivationFunctionType.Exp
    MUL = mybir.AluOpType.mult
    ADD = mybir.AluOpType.add

    nc = bacc.Bacc(None, target_bir_lowering=False)
    xT_d = nc.dram_tensor("xT", [FCH, 128, T], bf16, kind="ExternalInput")
    wqkvT_d = nc.dram_tensor("wqkvT", [FCH, 128, 3 * 512], bf16, kind="ExternalInput")
    woutT_d = nc.dram_tensor("woutT", [NPAIR, 128, D], bf16, kind="ExternalInput")
    bq_d = nc.dram_tensor("bq", [128, NPAIR], f32, kind="ExternalInput")
    bk_d = nc.dram_tensor("bk", [128, NPAIR], f32, kind="ExternalInput")
    kpb_d = nc.dram_tensor("kpb", [128, NKT], f32, kind="ExternalInput")
    mq_d = nc.dram_tensor("mq", [128, 5, 512], bf16, kind="ExternalInput")
    mk_d = nc.dram_tensor("mk", [128, 128], bf16, kind="ExternalInput")
    sel2_d = nc.dram_tensor("sel2", [2, 128], bf16, kind="ExternalInput")
    vones_d = nc.dram_tensor("vones", [128, NKT * HPG], bf16, kind="ExternalInput")
    out_d = nc.dram_tensor("out_part", [T, D], f32, kind="ExternalOutput")

    with tile.TileContext(nc) as tc, \
         nc.allow_low_precision(reason="bf16 matmul fast path"), \
         ExitStack() as top:
        pers = top.enter_context(tc.tile_pool(name="pers", bufs=1))
        xT_sb = pers.tile([128, FCH, T], bf16, name="xT_sb")
        wk_sb = pers.tile([128, FCH, 3 * 512], bf16, name="wk_sb")
        QT = pers.tile([128, NPAIR, T], bf16, name="QT")
        KT = pers.tile([128, NPAIR, T], bf16, name="KT")
        Vt = pers.tile([128, NKT, HPG, HD + 1], bf16, name="Vt")
        AT = pers.tile([128, NPAIR, T], bf16, name="AT")
        wo_sb = pers.tile([128, NPAIR, D], bf16, name="wo_sb")
        mq_sb = pers.tile([128, 5, 512], bf16, name="mq_sb")
        mk_sb = pers.tile([128, 128], bf16, name="mk_sb")
        kp_sb = pers.tile([128, NKT], f32, name="kp_sb")
        bq_sb = pers.tile([128, NPAIR], f32, name="bq_sb")
        bk_sb = pers.tile([128, NPAIR], f32, name="bk_sb")
        sel2_sb = pers.tile([2, 128], bf16, name="sel2_sb")
        vones_sb = pers.tile([128, NKT * HPG], bf16, name="vones_sb")
        # priority-sliced input loads: the f-major startup sweep needs, per f,
        # only the V weight columns, the QK weight columns, and the first
        # 1024 x columns -- stream those first (alternating both queues),
        # defer the upper x columns (used by fillers) and w_out
        nc.sync.dma_start(wk_sb[:, 0, 1024:1536], wqkvT_d[0, :, 1024:1536])
        nc.gpsimd.dma_start(xT_sb[:, 0, 0:1024], xT_d[0, :, 0:1024])
        nc.sync.dma_start(wk_sb[:, 0, 0:1024], wqkvT_d[0, :, 0:1024])
        nc.gpsimd.dma_start(mq_sb[:], mq_d[:])
        nc.gpsimd.dma_start(mk_sb[:], mk_d[:])
        nc.gpsimd.dma_start(kp_sb[:], kpb_d[:])
        nc.gpsimd.dma_start(bq_sb[:], bq_d[:])
        nc.gpsimd.dma_start(bk_sb[:], bk_d[:])
        nc.gpsimd.dma_start(sel2_sb[:], sel2_d[:])
        nc.gpsimd.dma_start(vones_sb[:], vones_d[:])
        for f in range(1, FCH):
            nc.sync.dma_start(wk_sb[:, f, :], wqkvT_d[f])
            nc.gpsimd.dma_start(xT_sb[:, f, 0:1024], xT_d[f, :, 0:1024])
        for f in range(FCH):
            (nc.sync if f % 2 else nc.gpsimd).dma_start(
                xT_sb[:, f, 1024:2048], xT_d[f, :, 1024:2048])
        for cchunk in range(NPAIR):
            (nc.sync if cchunk % 2 else nc.gpsimd).dma_start(
                wo_sb[:, cchunk, :], woutT_d[cchunk])
        nc.vector.tensor_copy(
            Vt[:, :, :, HD],
            vones_sb[:].rearrange("p (a b) -> p a b", a=NKT))

        def wQ(f, p):
            return wk_sb[:, f, 128 * p:128 * (p + 1)]

        def wK(f, p):
            return wk_sb[:, f, 512 + 128 * p:512 + 128 * (p + 1)]

        def wV(f):
            return wk_sb[:, f, 1024:1536]

        def v_copy(pv, t):
            nc.vector.tensor_copy(
                Vt[:, t, :, 0:HD],
                pv[:].rearrange("p (h d) -> p h d", h=HPG))

        def qk_store(pqk, tgt, t5):
            pair = tgt % 4
            dst = (QT if tgt < 4 else KT)[:, pair, t5 * 512:(t5 + 1) * 512]
            bias = (bq_sb if tgt < 4 else bk_sb)[:, pair:pair + 1]
            nc.vector.tensor_scalar(dst, pqk[:], bias, None, ADD)

        # ---- startup: f-major sweep (PE works while x^T still streams) ----
        with tc.tile_pool(name="psUp", bufs=1, space="PSUM") as psUp:
            pvs = [psUp.tile([128, 512], f32, name=f"pv{t}", tag=f"u{t}")
                   for t in range(5)]
            pk0 = psUp.tile([128, 512], f32, name="pk0", tag="u5")
            pk1 = psUp.tile([128, 512], f32, name="pk1", tag="u6")
            pq0 = psUp.tile([128, 512], f32, name="pq0", tag="u7")
            for f in range(FCH):
                st = dict(start=(f == 0), stop=(f == FCH - 1))
                for t in range(5):
                    nc.tensor.matmul(pvs[t][:], xT_sb[:, f, t * 128:(t + 1) * 128],
                                     wV(f), **st)
                nc.tensor.matmul(pk0[:], wK(f, 0), xT_sb[:, f, 0:512], **st)
                nc.tensor.matmul(pk1[:], wK(f, 0), xT_sb[:, f, 512:1024], **st)
                nc.tensor.matmul(pq0[:], wQ(f, 0), xT_sb[:, f, 0:512], **st)
            for t in range(5):
                v_copy(pvs[t], t)
            nc.vector.tensor_scalar(KT[:, 0, 0:512], pk0[:], bk_sb[:, 0:1], None, ADD)
            nc.vector.tensor_scalar(KT[:, 0, 512:1024], pk1[:], bk_sb[:, 0:1], None, ADD)
            nc.vector.tensor_scalar(QT[:, 0, 0:512], pq0[:], bq_sb[:, 0:1], None, ADD)

        # ---- attention (with projection filler inside block 0) ----
        with tc.tile_pool(name="eps", bufs=3) as epool, \
             tc.tile_pool(name="nsb", bufs=1) as nsb, \
             tc.tile_pool(name="avp", bufs=1) as avp, \
             tc.tile_pool(name="osb", bufs=2) as osb, \
             tc.tile_pool(name="psAv", bufs=1, space="PSUM") as psAv, \
             tc.tile_pool(name="psSc", bufs=2, space="PSUM") as psSc, \
             tc.tile_pool(name="psX", bufs=1, space="PSUM") as psX:

            nx = [0]

            def emitQK(tgt, t5):
                pqk = psX.tile([128, 512], f32, name="pqk", tag=f"x{nx[0] % 2}")
                nx[0] += 1
                w = wQ if tgt < 4 else wK
                for f in range(FCH):
                    nc.tensor.matmul(pqk[:], w(f, tgt % 4),
                                     xT_sb[:, f, t5 * 512:(t5 + 1) * 512],
                                     start=(f == 0), stop=(f == FCH - 1))
                qk_store(pqk, tgt, t5)

            def emitV(t):
                pv = psX.tile([128, 512], f32, name="pv", tag=f"x{nx[0] % 2}")
                nx[0] += 1
                for f in range(FCH):
                    nc.tensor.matmul(pv[:], xT_sb[:, f, t * 128:(t + 1) * 128],
                                     wV(f), start=(f == 0), stop=(f == FCH - 1))
                v_copy(pv, t)

            # remaining startup QK blocks (t5-major, 2-bank pipeline)
            rest = [(4, 2), (4, 3), (0, 1), (0, 2), (0, 3)]
            fillers = [
                [("V", t) for t in range(5, 9)] + [(5, t5) for t5 in range(4)]
                + [(1, t5) for t5 in range(4)],
                [("V", t) for t in range(9, 13)] + [(6, t5) for t5 in range(4)]
                + [(2, t5) for t5 in range(4)],
                [("V", t) for t in range(13, 16)] + [(7, t5) for t5 in range(4)]
                + [(3, t5) for t5 in range(4)],
                [],
            ]
            for tgt, t5 in rest:
                emitQK(tgt, t5)

            norm_pend = {}

            def emit_normalize_pair(q5, p):
                av2, recp = norm_pend.pop((q5, p))
                qs = slice(q5 * 512, (q5 + 1) * 512)
                bc = psX.tile([128, 512], f32, name="bc", tag="x0")
                if isinstance(recp, tuple):
                    # tail fast path: two per-head broadcasts from separate
                    # [1, 512] reciprocal rows
                    nc.tensor.matmul(bc[0:64, :], sel2_sb[0:1, 0:64],
                                     recp[0][:], start=True, stop=True,
                                     tile_position=(0, 0))
                    nc.tensor.matmul(bc[64:128, :], sel2_sb[0:1, 0:64],
                                     recp[1][:], start=True, stop=True,
                                     tile_position=(0, 64))
                else:
                    nc.tensor.matmul(bc[:], sel2_sb[:], recp[:],
                                     start=True, stop=True)
                nc.vector.tensor_tensor(AT[0:64, p, qs], av2[0:64, 0, :],
                                        bc[0:64, :], MUL)
                nc.vector.tensor_tensor(AT[64:128, p, qs], av2[0:64, 1, :],
                                        bc[64:128, :], MUL)

            def emit_proj(q5):
                for tq in range(4):
                    t = 4 * q5 + tq
                    tsl = slice(t * 128, (t + 1) * 128)
                    po0 = psX.tile([128, 512], f32, name="po0", tag="x0")
                    po1 = psX.tile([128, 512], f32, name="po1", tag="x1")
                    for cchunk in range(NPAIR):
                        lhsT = AT[:, cchunk, tsl]
                        nc.tensor.matmul(po0[:], lhsT, wo_sb[:, cchunk, 0:512],
                                         start=(cchunk == 0), stop=(cchunk == 3))
                        nc.tensor.matmul(po1[:], lhsT, wo_sb[:, cchunk, 512:1024],
                                         start=(cchunk == 0), stop=(cchunk == 3))
                    ot = osb.tile([128, D], f32, name="ot", tag="ot")
                    nc.vector.tensor_copy(ot[:, 0:512], po0[:])
                    nc.vector.tensor_copy(ot[:, 512:1024], po1[:])
                    (nc.sync if t % 2 else nc.gpsimd).dma_start(
                        out_d[tsl, :], ot[:])

            # Flat software pipeline across pairs/blocks: each pair's last AV
            # is deferred past the next pair's first score group so the PE
            # never drains while ScalarE finishes the last exp.
            pend_av = [None]

            def flush_av():
                if pend_av[0] is not None:
                    pend_av[0]()
                    pend_av[0] = None

            def finish_pair(q5, p, avA, avB):
                # AV out of PSUM into one combined tile (slot-freeing copies
                # first), then both denominator rows gathered with a single
                # SBUF->SBUF DMA (DVE can't address partition base 1) for one
                # batched reciprocal + cast
                av2 = avp.tile([HD + 1, 2, 512], f32,
                               name=f"av2_{p}", tag=f"avp{p}")
                nc.vector.tensor_copy(av2[:, 0, :], avA[:])
                nc.vector.tensor_copy(av2[:, 1, :], avB[:])
                d2 = nsb.tile([2, 512], f32, name="d2", tag=f"dp{p}")
                nc.gpsimd.dma_start(d2[:], av2[64:65, :, :])
                rc32 = nsb.tile([2, 512], f32, name="rc32", tag=f"di{p}")
                nc.vector.reciprocal_approx_fast(rc32[:], d2[:])
                recp = nsb.tile([2, 512], bf16, name="recp", tag=f"rc{p}")
                nc.gpsimd.tensor_copy(recp[:], rc32[:])
                norm_pend[(q5, p)] = (av2, recp)

            for q5 in range(TQ5):
                nkt = min(4 * q5 + 5, NKT)
                q5s = q5 * 512
                for p in range(NPAIR):
                    avA = psAv.tile([HD + 1, 512], f32, name="avA", tag="avA")
                    avB = psAv.tile([HD + 1, 512], f32, name="avB", tag="avB")
                    for kt in range(nkt):
                        ks = slice(kt * 128, (kt + 1) * 128)
                        off = kt - 4 * q5
                        masked = off >= 0
                        # masked tiles only affect queries >= q0
                        q0 = max(0, 128 * off - L) if masked else 0
                        qs = slice(q5s + q0, q5s + 512)
                        sc2 = psSc.tile([128, 2, 512], f32, name="sc2", tag="sc2")
                        nc.tensor.matmul(sc2[:, 0, q0:512],
                                         KT[0:64, p, ks], QT[0:64, p, qs],
                                         start=True, stop=not masked,
                                         tile_position=(0, 0))
                        nc.tensor.matmul(sc2[:, 1, q0:512],
                                         KT[64:128, p, ks], QT[64:128, p, qs],
                                         start=True, stop=not masked,
                                         tile_position=(64, 0))
                        if masked:
                            m1 = min(512, 128 * off + 126)
                            nc.tensor.matmul(sc2[:, 0, q0:m1], mk_sb[:],
                                             mq_sb[:, off, q0:m1],
                                             start=False, stop=True,
                                             skip_group_check=True)
                            nc.tensor.matmul(sc2[:, 1, q0:m1], mk_sb[:],
                                             mq_sb[:, off, q0:m1],
                                             start=False, stop=True,
                                             skip_group_check=True)
                        flush_av()
                        if kt == 1:
                            # deferred bookkeeping once the pipeline is primed:
                            # block-delayed normalize of (q5-1, p), plus
                            # pair-delayed normalize inside the last block
                            if q5 >= 1 and (q5 - 1, p) in norm_pend:
                                emit_normalize_pair(q5 - 1, p)
                                if p == 3:
                                    emit_proj(q5 - 1)
                        if kt == 2 and q5 == TQ5 - 1 and p >= 1 and \
                                (q5, p - 1) in norm_pend:
                            emit_normalize_pair(q5, p - 1)
                        e2 = epool.tile([128, 2, 512], bf16, name="e2", tag="e2")
                        nc.scalar.activation(e2[:, :, q0:512], sc2[:, :, q0:512],
                                             Exp, bias=kp_sb[:, kt:kt + 1],
                                             scale=1.0 / math.sqrt(HD))

                        def mk_av(kt=kt, e2=e2, q0=q0, avA=avA, avB=avB,
                                  p=p, nkt=nkt, q5=q5):
                            nc.tensor.matmul(avA[0:65, q0:512],
                                             Vt[:, kt, 2 * p, :],
                                             e2[:, 0, q0:512],
                                             start=(kt == 0), stop=(kt == nkt - 1),
                                             skip_group_check=True)
                            nc.tensor.matmul(avB[0:65, q0:512],
                                             Vt[:, kt, 2 * p + 1, :],
                                             e2[:, 1, q0:512],
                                             start=(kt == 0), stop=(kt == nkt - 1),
                                             skip_group_check=True)
                            if kt == nkt - 1:
                                finish_pair(q5, p, avA, avB)
                        pend_av[0] = mk_av
                    # projection filler inside block 0 keeps the PE warm
                    if q5 == 0:
                        for j, item in enumerate(fillers[p]):
                            if item[0] == "V":
                                emitV(item[1])
                            else:
                                emitQK(item[0], item[1])
                            if j == 0:
                                flush_av()
            # drain: last pair's AV, its normalize, last projection
            flush_av()
            emit_normalize_pair(TQ5 - 1, 3)
            emit_proj(TQ5 - 1)

    nc.finalize()
    return nc


def _host_inputs(x, key_padding_mask, w_qkv, b_qkv, w_out):
    """Per-core input dicts."""
    import ml_dtypes

    f32 = np.float32
    bf = ml_dtypes.bfloat16
    # masks (shared across cores)
    j = np.arange(128)[:, None]
    q = np.arange(512)[None, :]
    mq = np.zeros((128, 5, 512), f32)
    for off in range(5):
        mq[:, off, :] = (128 * off + j > q + L).astype(f32)
    mq = mq.astype(bf)
    mk = (NEG * np.eye(128, dtype=f32)).astype(bf)
    vones = np.ones((128, NKT * HPG), bf)
    sel2 = np.zeros((2, 128), f32)
    sel2[0, 0:64] = 1.0
    sel2[1, 64:128] = 1.0
    sel2 = sel2.astype(bf)

    in_maps = []
    for c in range(NCORES):
        b, g = divmod(c, 2)
        # channel rows for this group's Q/K (pairs of heads -> 128 rows each)
        qrows = np.concatenate(
            [w_qkv[64 * (8 * g + 2 * p):64 * (8 * g + 2 * p) + 128] for p in range(NPAIR)])
        krows = np.concatenate(
            [w_qkv[D + 64 * (8 * g + 2 * p):D + 64 * (8 * g + 2 * p) + 128] for p in range(NPAIR)])
        vrows = w_qkv[2 * D + 512 * g:2 * D + 512 * g + 512]
        w_all = np.concatenate([qrows, krows, vrows], 0)          # [1536, 1024]
        wqkvT = np.ascontiguousarray(w_all.T).reshape(FCH, 128, 3 * 512)
        bq = np.stack(
            [b_qkv[64 * (8 * g + 2 * p):64 * (8 * g + 2 * p) + 128] for p in range(NPAIR)], 1)
        bk = np.stack(
            [b_qkv[D + 64 * (8 * g + 2 * p):D + 64 * (8 * g + 2 * p) + 128] for p in range(NPAIR)], 1)
        xT = np.ascontiguousarray(x[b].T).reshape(FCH, 128, T)
        woutT = np.ascontiguousarray(w_out.T[512 * g:512 * g + 512]).reshape(NPAIR, 128, D)
        kpb = np.ascontiguousarray(
            (NEG * key_padding_mask[b].astype(f32)).reshape(NKT, 128).T)
        in_maps.append({
            "xT": xT.astype(bf), "wqkvT": wqkvT.astype(bf),
            "woutT": woutT.astype(bf),
            "bq": bq.astype(f32), "bk": bk.astype(f32), "kpb": kpb.astype(f32),
            "mq": mq, "mk": mk, "vones": vones, "sel2": sel2,
        })
    return in_maps


def kernel(x, key_padding_mask, w_qkv, b_qkv, w_out, b_out):
    from concourse.bass_utils import run_bass_kernel_spmd

    x = np.asarray(x, np.float32)
    key_padding_mask = np.asarray(key_padding_mask)
    w_qkv = np.asarray(w_qkv, np.float32)
    b_qkv = np.asarray(b_qkv, np.float32)
    w_out = np.asarray(w_out, np.float32)
    b_out = np.asarray(b_out, np.float32)

    if "nc" not in _BUILT:
        _BUILT["nc"] = _build_nc()
    nc = _BUILT["nc"]

    in_maps = _host_inputs(x, key_padding_mask, w_qkv, b_qkv, w_out)
    res = run_bass_kernel_spmd(nc, in_maps, core_ids=list(range(NCORES)))
    out = np.empty((B, T, D), np.float32)
    for b in range(B):
        out[b] = res.results[2 * b]["out_part"] + res.results[2 * b + 1]["out_part"]
    # host-folded biases: b_out plus the V-bias pushed through the projection
    bv = b_qkv[2 * D:3 * D]
    out += (b_out + bv @ w_out.T)[None, None, :].astype(np.float32)
    return out


# revision 37
# speedup vs baseline: 1.0854x; 1.0028x over previous
"""CT self-attention (causal + 2 future frames) for Trainium2, 8 NeuronCores.

Sharding: batch (4-way) x head-group (2-way): core c = 2*b + g handles batch b,
heads [8g, 8g+8). Each core computes its QKV projection slice, banded
attention for its 8 heads, and a partial output projection; the host sums the
two partial outputs per batch and adds the (host-folded) biases.

All matmuls run in bf16 (1 col/cycle @ 2.4 GHz warm) with fp32 PSUM.
Schedule is built for PE density (HAM stays at full clock):
  - startup: f-major sweep accumulating V t0-4 + K0 (first 2 blocks) + Q0
    (block 0) in 8 PSUM banks while the x^T / w DMAs stream in, then the
    remaining K0/Q0 blocks
  - attention for block q5=0 interleaves the rest of the QKV projection
    (V t5-15, K1-3, Q1-3) between head pairs as PE gap filler
  - attention: scores S_T = K^T-tile.T @ Q (2 heads row-tiled), CT mask via
    accumulating -1e9*I @ MQ matmul, exp on ScalarE (scale 1/8, key-padding
    bias), AV with a ones column on V (denominator on partition 64)
  - normalize per head pair: 2 denominator rows -> [2,512] fast reciprocal,
    one [2,128] selector matmul broadcasts both heads' reciprocals across
    128 partitions, 2 DVE mults into AT; deferred into the next block's
    stream (immediate for the last block)
  - output projection per 128-query tile once all 4 pairs are normalized
"""
import math
from contextlib import ExitStack

import numpy as np

B, T, D, H = 4, 2048, 1024, 16
HD = D // H            # 64
L = 2                  # max_future_frames
NCORES = 8
HPG = 8                # heads per group/core
NPAIR = 4              # head pairs per core
FCH = 8                # feature chunks (D / 128)
TQ5 = 4                # 512-wide query tiles
NKT = 16               # 128-wide key tiles
NEG = -1.0e9

_BUILT = {}


def _build_nc():
    import concourse.tile as tile
    from concourse import bacc, mybir

    dt = mybir.dt
    f32, f32r, bf16 = dt.float32, dt.float32r, dt.bfloat16
    Exp = mybir.ActivationFunctionType.Exp
    MUL = mybir.AluOpType.mult
    ADD = mybir.AluOpType.add

    nc = bacc.Bacc(None, target_bir_lowering=False)
    xT_d = nc.dram_tensor("xT", [FCH, 128, T], bf16, kind="ExternalInput")
    wqkvT_d = nc.dram_tensor("wqkvT", [FCH, 128, 3 * 512], bf16, kind="ExternalInput")
    woutT_d = nc.dram_tensor("woutT", [NPAIR, 128, D], bf16, kind="ExternalInput")
    bq_d = nc.dram_tensor("bq", [128, NPAIR], f32, kind="ExternalInput")
    bk_d = nc.dram_tensor("bk", [128, NPAIR], f32, kind="ExternalInput")
    kpb_d = nc.dram_tensor("kpb", [128, NKT], f32, kind="ExternalInput")
    mq_d = nc.dram_tensor("mq", [128, 5, 512], bf16, kind="ExternalInput")
    mk_d = nc.dram_tensor("mk", [128, 128], bf16, kind="ExternalInput")
    sel2_d = nc.dram_tensor("sel2", [2, 128], bf16, kind="ExternalInput")
    vones_d = nc.dram_tensor("vones", [128, NKT * HPG], bf16, kind="ExternalInput")
    out_d = nc.dram_tensor("out_part", [T, D], f32, kind="ExternalOutput")

    with tile.TileContext(nc) as tc, \
         nc.allow_low_precision(reason="bf16 matmul fast path"), \
         ExitStack() as top:
        pers = top.enter_context(tc.tile_pool(name="pers", bufs=1))
        xT_sb = pers.tile([128, FCH, T], bf16, name="xT_sb")
        wk_sb = pers.tile([128, FCH, 3 * 512], bf16, name="wk_sb")
        QT = pers.tile([128, NPAIR, T], bf16, name="QT")
        KT = pers.tile([128, NPAIR, T], bf16, name="KT")
        Vt = pers.tile([128, NKT, HPG, HD + 1], bf16, name="Vt")
        AT = pers.tile([128, NPAIR, T], bf16, name="AT")
        wo_sb = pers.tile([128, NPAIR, D], bf16, name="wo_sb")
        mq_sb = pers.tile([128, 5, 512], bf16, name="mq_sb")
        mk_sb = pers.tile([128, 128], bf16, name="mk_sb")
        kp_sb = pers.tile([128, NKT], f32, name="kp_sb")
        bq_sb = pers.tile([128, NPAIR], f32, name="bq_sb")
        bk_sb = pers.tile([128, NPAIR], f32, name="bk_sb")
        sel2_sb = pers.tile([2, 128], bf16, name="sel2_sb")
        vones_sb = pers.tile([128, NKT * HPG], bf16, name="vones_sb")
        # priority-sliced input loads: the f-major startup sweep needs, per f,
        # only the [Q0|K0|V] weight slice and the first 1024 x columns --
        # stream those first (split across both queues), defer the remaining
        # weight pairs (fillers), the upper x columns, and w_out
        nc.sync.dma_start(wk_sb[:, 0, 0:768], wqkvT_d[0, :, 0:768])
        nc.gpsimd.dma_start(xT_sb[:, 0, 0:1024], xT_d[0, :, 0:1024])
        nc.gpsimd.dma_start(mq_sb[:], mq_d[:])
        nc.gpsimd.dma_start(mk_sb[:], mk_d[:])
        nc.gpsimd.dma_start(kp_sb[:], kpb_d[:])
        nc.gpsimd.dma_start(bq_sb[:], bq_d[:])
        nc.gpsimd.dma_start(bk_sb[:], bk_d[:])
        nc.gpsimd.dma_start(sel2_sb[:], sel2_d[:])
        nc.gpsimd.dma_start(vones_sb[:], vones_d[:])
        for f in range(1, FCH):
            nc.sync.dma_start(wk_sb[:, f, 0:768], wqkvT_d[f, :, 0:768])
            nc.gpsimd.dma_start(xT_sb[:, f, 0:1024], xT_d[f, :, 0:1024])
        for f in range(FCH):
            (nc.sync if f % 2 else nc.gpsimd).dma_start(
                wk_sb[:, f, 768:1536], wqkvT_d[f, :, 768:1536])
        for f in range(FCH):
            (nc.sync if f % 2 else nc.gpsimd).dma_start(
                xT_sb[:, f, 1024:2048], xT_d[f, :, 1024:2048])
        for cchunk in range(NPAIR):
            (nc.sync if cchunk % 2 else nc.gpsimd).dma_start(
                wo_sb[:, cchunk, :], woutT_d[cchunk])
        nc.vector.tensor_copy(
            Vt[:, :, :, HD],
            vones_sb[:].rearrange("p (a b) -> p a b", a=NKT))

        # host packs the weight columns as [Q0 | K0 | V | Q1-3 | K1-3] so the
        # startup sweep's needs are one contiguous priority DMA slice
        def wQ(f, p):
            return wk_sb[:, f, 0:128] if p == 0 else \
                wk_sb[:, f, 768 + 128 * (p - 1):768 + 128 * p]

        def wK(f, p):
            return wk_sb[:, f, 128:256] if p == 0 else \
                wk_sb[:, f, 1152 + 128 * (p - 1):1152 + 128 * p]

        def wV(f):
            return wk_sb[:, f, 256:768]

        def v_copy(pv, t):
            nc.vector.tensor_copy(
                Vt[:, t, :, 0:HD],
                pv[:].rearrange("p (h d) -> p h d", h=HPG))

        def qk_store(pqk, tgt, t5):
            pair = tgt % 4
            dst = (QT if tgt < 4 else KT)[:, pair, t5 * 512:(t5 + 1) * 512]
            bias = (bq_sb if tgt < 4 else bk_sb)[:, pair:pair + 1]
            nc.vector.tensor_scalar(dst, pqk[:], bias, None, ADD)

        # ---- startup: f-major sweep (PE works while x^T still streams) ----
        with tc.tile_pool(name="psUp", bufs=1, space="PSUM") as psUp:
            pvs = [psUp.tile([128, 512], f32, name=f"pv{t}", tag=f"u{t}")
                   for t in range(5)]
            pk0 = psUp.tile([128, 512], f32, name="pk0", tag="u5")
            pk1 = psUp.tile([128, 512], f32, name="pk1", tag="u6")
            pq0 = psUp.tile([128, 512], f32, name="pq0", tag="u7")
            for f in range(FCH):
                st = dict(start=(f == 0), stop=(f == FCH - 1))
                for t in range(5):
                    nc.tensor.matmul(pvs[t][:], xT_sb[:, f, t * 128:(t + 1) * 128],
                                     wV(f), **st)
                nc.tensor.matmul(pk0[:], wK(f, 0), xT_sb[:, f, 0:512], **st)
                nc.tensor.matmul(pk1[:], wK(f, 0), xT_sb[:, f, 512:1024], **st)
                nc.tensor.matmul(pq0[:], wQ(f, 0), xT_sb[:, f, 0:512], **st)
            for t in range(5):
                v_copy(pvs[t], t)
            nc.vector.tensor_scalar(KT[:, 0, 0:512], pk0[:], bk_sb[:, 0:1], None, ADD)
            nc.vector.tensor_scalar(KT[:, 0, 512:1024], pk1[:], bk_sb[:, 0:1], None, ADD)
            nc.vector.tensor_scalar(QT[:, 0, 0:512], pq0[:], bq_sb[:, 0:1], None, ADD)

        # ---- attention (with projection filler inside block 0) ----
        with tc.tile_pool(name="eps", bufs=3) as epool, \
             tc.tile_pool(name="nsb", bufs=1) as nsb, \
             tc.tile_pool(name="avp", bufs=1) as avp, \
             tc.tile_pool(name="osb", bufs=2) as osb, \
             tc.tile_pool(name="psAv", bufs=1, space="PSUM") as psAv, \
             tc.tile_pool(name="psSc", bufs=2, space="PSUM") as psSc, \
             tc.tile_pool(name="psX", bufs=1, space="PSUM") as psX:

            nx = [0]

            def emitQK(tgt, t5):
                pqk = psX.tile([128, 512], f32, name="pqk", tag=f"x{nx[0] % 2}")
                nx[0] += 1
                w = wQ if tgt < 4 else wK
                for f in range(FCH):
                    nc.tensor.matmul(pqk[:], w(f, tgt % 4),
                                     xT_sb[:, f, t5 * 512:(t5 + 1) * 512],
                                     start=(f == 0), stop=(f == FCH - 1))
                qk_store(pqk, tgt, t5)

            def emitV(t):
                pv = psX.tile([128, 512], f32, name="pv", tag=f"x{nx[0] % 2}")
                nx[0] += 1
                for f in range(FCH):
                    nc.tensor.matmul(pv[:], xT_sb[:, f, t * 128:(t + 1) * 128],
                                     wV(f), start=(f == 0), stop=(f == FCH - 1))
                v_copy(pv, t)

            # remaining startup QK blocks (t5-major, 2-bank pipeline)
            rest = [(4, 2), (4, 3), (0, 1), (0, 2), (0, 3)]
            fillers = [
                [("V", t) for t in range(5, 9)] + [(5, t5) for t5 in range(4)]
                + [(1, t5) for t5 in range(4)],
                [("V", t) for t in range(9, 13)] + [(6, t5) for t5 in range(4)]
                + [(2, t5) for t5 in range(4)],
                [("V", t) for t in range(13, 16)] + [(7, t5) for t5 in range(4)]
                + [(3, t5) for t5 in range(4)],
                [],
            ]
            for tgt, t5 in rest:
                emitQK(tgt, t5)

            norm_pend = {}

            def emit_normalize_pair(q5, p):
                av2, recp = norm_pend.pop((q5, p))
                qs = slice(q5 * 512, (q5 + 1) * 512)
                bc = psX.tile([128, 512], f32, name="bc", tag="x0")
                if isinstance(recp, tuple):
                    # tail fast path: two per-head broadcasts from separate
                    # [1, 512] reciprocal rows
                    nc.tensor.matmul(bc[0:64, :], sel2_sb[0:1, 0:64],
                                     recp[0][:], start=True, stop=True,
                                     tile_position=(0, 0))
                    nc.tensor.matmul(bc[64:128, :], sel2_sb[0:1, 0:64],
                                     recp[1][:], start=True, stop=True,
                                     tile_position=(0, 64))
                else:
                    nc.tensor.matmul(bc[:], sel2_sb[:], recp[:],
                                     start=True, stop=True)
                nc.vector.tensor_tensor(AT[0:64, p, qs], av2[0:64, 0, :],
                                        bc[0:64, :], MUL)
                nc.vector.tensor_tensor(AT[64:128, p, qs], av2[0:64, 1, :],
                                        bc[64:128, :], MUL)

            def emit_proj(q5):
                for tq in range(4):
                    t = 4 * q5 + tq
                    tsl = slice(t * 128, (t + 1) * 128)
                    po0 = psX.tile([128, 512], f32, name="po0", tag="x0")
                    po1 = psX.tile([128, 512], f32, name="po1", tag="x1")
                    for cchunk in range(NPAIR):
                        lhsT = AT[:, cchunk, tsl]
                        nc.tensor.matmul(po0[:], lhsT, wo_sb[:, cchunk, 0:512],
                                         start=(cchunk == 0), stop=(cchunk == 3))
                        nc.tensor.matmul(po1[:], lhsT, wo_sb[:, cchunk, 512:1024],
                                         start=(cchunk == 0), stop=(cchunk == 3))
                    ot = osb.tile([128, D], f32, name="ot", tag="ot")
                    nc.vector.tensor_copy(ot[:, 0:512], po0[:])
                    nc.vector.tensor_copy(ot[:, 512:1024], po1[:])
                    (nc.sync if t % 2 else nc.gpsimd).dma_start(
                        out_d[tsl, :], ot[:])

            # Flat software pipeline across pairs/blocks: each pair's last AV
            # is deferred past the next pair's first score group so the PE
            # never drains while ScalarE finishes the last exp.
            pend_av = [None]

            def flush_av():
                if pend_av[0] is not None:
                    pend_av[0]()
                    pend_av[0] = None

            def finish_pair(q5, p, avA, avB):
                # AV out of PSUM into one combined tile (slot-freeing copies
                # first), then both denominator rows gathered with a single
                # SBUF->SBUF DMA (DVE can't address partition base 1) for one
                # batched reciprocal + cast
                av2 = avp.tile([HD + 1, 2, 512], f32,
                               name=f"av2_{p}", tag=f"avp{p}")
                nc.vector.tensor_copy(av2[:, 0, :], avA[:])
                nc.vector.tensor_copy(av2[:, 1, :], avB[:])
                d2 = nsb.tile([2, 512], f32, name="d2", tag=f"dp{p}")
                nc.gpsimd.dma_start(d2[:], av2[64:65, :, :])
                rc32 = nsb.tile([2, 512], f32, name="rc32", tag=f"di{p}")
                nc.vector.reciprocal_approx_fast(rc32[:], d2[:])
                recp = nsb.tile([2, 512], bf16, name="recp", tag=f"rc{p}")
                nc.gpsimd.tensor_copy(recp[:], rc32[:])
                norm_pend[(q5, p)] = (av2, recp)

            for q5 in range(TQ5):
                nkt = min(4 * q5 + 5, NKT)
                q5s = q5 * 512
                for p in range(NPAIR):
                    avA = psAv.tile([HD + 1, 512], f32, name="avA", tag="avA")
                    avB = psAv.tile([HD + 1, 512], f32, name="avB", tag="avB")
                    for kt in range(nkt):
                        ks = slice(kt * 128, (kt + 1) * 128)
                        off = kt - 4 * q5
                        masked = off >= 0
                        # masked tiles only affect queries >= q0
                        q0 = max(0, 128 * off - L) if masked else 0
                        qs = slice(q5s + q0, q5s + 512)
                        sc2 = psSc.tile([128, 2, 512], f32, name="sc2", tag="sc2")
                        nc.tensor.matmul(sc2[:, 0, q0:512],
                                         KT[0:64, p, ks], QT[0:64, p, qs],
                                         start=True, stop=not masked,
                                         tile_position=(0, 0))
                        nc.tensor.matmul(sc2[:, 1, q0:512],
                                         KT[64:128, p, ks], QT[64:128, p, qs],
                                         start=True, stop=not masked,
                                         tile_position=(64, 0))
                        if masked:
                            m1 = min(512, 128 * off + 126)
                            nc.tensor.matmul(sc2[:, 0, q0:m1], mk_sb[:],
                                             mq_sb[:, off, q0:m1],
                                             start=False, stop=True,
                                             skip_group_check=True)
                            nc.tensor.matmul(sc2[:, 1, q0:m1], mk_sb[:],
                                             mq_sb[:, off, q0:m1],
                                             start=False, stop=True,
                                             skip_group_check=True)
                        flush_av()
                        if kt == 1:
                            # deferred bookkeeping once the pipeline is primed:
                            # block-delayed normalize of (q5-1, p), plus
                            # pair-delayed normalize inside the last block
                            if q5 >= 1 and (q5 - 1, p) in norm_pend:
                                emit_normalize_pair(q5 - 1, p)
                                if p == 3:
                                    emit_proj(q5 - 1)
                        if kt == 2 and q5 == TQ5 - 1 and p >= 1 and \
                                (q5, p - 1) in norm_pend:
                            emit_normalize_pair(q5, p - 1)
                        e2 = epool.tile([128, 2, 512], bf16, name="e2", tag="e2")
                        nc.scalar.activation(e2[:, :, q0:512], sc2[:, :, q0:512],
                                             Exp, bias=kp_sb[:, kt:kt + 1],
                                             scale=1.0 / math.sqrt(HD))

                        def mk_av(kt=kt, e2=e2, q0=q0, avA=avA, avB=avB,
                                  p=p, nkt=nkt, q5=q5):
                            nc.tensor.matmul(avA[0:65, q0:512],
                                             Vt[:, kt, 2 * p, :],
                                             e2[:, 0, q0:512],
                                             start=(kt == 0), stop=(kt == nkt - 1),
                                             skip_group_check=True)
                            nc.tensor.matmul(avB[0:65, q0:512],
                                             Vt[:, kt, 2 * p + 1, :],
                                             e2[:, 1, q0:512],
                                             start=(kt == 0), stop=(kt == nkt - 1),
                                             skip_group_check=True)
                            if kt == nkt - 1:
                                finish_pair(q5, p, avA, avB)
                        pend_av[0] = mk_av
                    # projection filler inside block 0 keeps the PE warm
                    if q5 == 0:
                        for j, item in enumerate(fillers[p]):
                            if item[0] == "V":
                                emitV(item[1])
                            else:
                                emitQK(item[0], item[1])
                            if j == 0:
                                flush_av()
            # drain: last pair's AV, its normalize, last projection
            flush_av()
            emit_normalize_pair(TQ5 - 1, 3)
            emit_proj(TQ5 - 1)

    nc.finalize()
    return nc


def _host_inputs(x, key_padding_mask, w_qkv, b_qkv, w_out):
    """Per-core input dicts."""
    import ml_dtypes

    f32 = np.float32
    bf = ml_dtypes.bfloat16
    # masks (shared across cores)
    j = np.arange(128)[:, None]
    q = np.arange(512)[None, :]
    mq = np.zeros((128, 5, 512), f32)
    for off in range(5):
        mq[:, off, :] = (128 * off + j > q + L).astype(f32)
    mq = mq.astype(bf)
    mk = (NEG * np.eye(128, dtype=f32)).astype(bf)
    vones = np.ones((128, NKT * HPG), bf)
    sel2 = np.zeros((2, 128), f32)
    sel2[0, 0:64] = 1.0
    sel2[1, 64:128] = 1.0
    sel2 = sel2.astype(bf)

    in_maps = []
    for c in range(NCORES):
        b, g = divmod(c, 2)
        # channel rows for this group's Q/K (pairs of heads -> 128 rows each)
        qrows = np.concatenate(
            [w_qkv[64 * (8 * g + 2 * p):64 * (8 * g + 2 * p) + 128] for p in range(NPAIR)])
        krows = np.concatenate(
            [w_qkv[D + 64 * (8 * g + 2 * p):D + 64 * (8 * g + 2 * p) + 128] for p in range(NPAIR)])
        vrows = w_qkv[2 * D + 512 * g:2 * D + 512 * g + 512]
        # column order [Q0 | K0 | V | Q1-3 | K1-3]: the startup sweep's
        # weights form one contiguous priority DMA slice
        w_all = np.concatenate([qrows[0:128], krows[0:128], vrows,
                                qrows[128:512], krows[128:512]], 0)
        wqkvT = np.ascontiguousarray(w_all.T).reshape(FCH, 128, 3 * 512)
        bq = np.stack(
            [b_qkv[64 * (8 * g + 2 * p):64 * (8 * g + 2 * p) + 128] for p in range(NPAIR)], 1)
        bk = np.stack(
            [b_qkv[D + 64 * (8 * g + 2 * p):D + 64 * (8 * g + 2 * p) + 128] for p in range(NPAIR)], 1)
        xT = np.ascontiguousarray(x[b].T).reshape(FCH, 128, T)
        woutT = np.ascontiguousarray(w_out.T[512 * g:512 * g + 512]).reshape(NPAIR, 128, D)
        kpb = np.ascontiguousarray(
            (NEG * key_padding_mask[b].astype(f32)).reshape(NKT, 128).T)
        in_maps.append({
            "xT": xT.astype(bf), "wqkvT": wqkvT.astype(bf),
            "woutT": woutT.astype(bf),
            "bq": bq.astype(f32), "bk": bk.astype(f32), "kpb": kpb.astype(f32),
            "mq": mq, "mk": mk, "vones": vones, "sel2": sel2,
        })
    return in_maps


def kernel(x, key_padding_mask, w_qkv, b_qkv, w_out, b_out):
    from concourse.bass_utils import run_bass_kernel_spmd

    x = np.asarray(x, np.float32)
    key_padding_mask = np.asarray(key_padding_mask)
    w_qkv = np.asarray(w_qkv, np.float32)
    b_qkv = np.asarray(b_qkv, np.float32)
    w_out = np.asarray(w_out, np.float32)
    b_out = np.asarray(b_out, np.float32)

    if "nc" not in _BUILT:
        _BUILT["nc"] = _build_nc()
    nc = _BUILT["nc"]

    in_maps = _host_inputs(x, key_padding_mask, w_qkv, b_qkv, w_out)
    res = run_bass_kernel_spmd(nc, in_maps, core_ids=list(range(NCORES)))
    out = np.empty((B, T, D), np.float32)
    for b in range(B):
        out[b] = res.results[2 * b]["out_part"] + res.results[2 * b + 1]["out_part"]
    # host-folded biases: b_out plus the V-bias pushed through the projection
    bv = b_qkv[2 * D:3 * D]
    out += (b_out + bv @ w_out.T)[None, None, :].astype(np.float32)
    return out
